# revision 19
# baseline (speedup 1.0000x reference)
"""BertAttention (single-head, H=768, S=2048, B=8) for 8 Trainium2 NeuronCores.

Data-parallel over batch: each core computes one batch element end to end
(QKV projections, masked softmax attention, output projection, residual,
LayerNorm). No collectives.

Layout strategy (all on-chip intermediates keep the contraction dim on
partitions so no transposes of intermediates are ever needed):
    Y   = X^T        [768, 2048]  (PE transposes of X tiles)
    Q^T = Wq^T as lhsT? no:  Q^T[h,s] = sum_k Wq[k,h] Y[k,s]   lhsT=Wq, rhs=Y
    K^T same; V[t,h] = sum_k Y[k,t-tile]^T ... lhsT=Y, rhs=Wv
    S^T[t,s] = sum_h K^T[h,t] Q^T[h,s]        lhsT=K^T, rhs=Q^T
    P^T = exp(S^T*scale + maskoff[t])          (ACT eviction; masked -> 0.0)
    Z[s] = sum_t P^T[t,s]                      lhsT=P^T slice, rhs=ones
    ctx^T[h,s] = sum_t V[t,h] P^T[t,s]         lhsT=V, rhs=P^T
    O[s,h'] = sum_h ctx^T[h,s] Wo[h,h']        lhsT=ctx^T, rhs=Wo
    h = O/Z + X ; LayerNorm(h) via bn_stats/bn_aggr.

Matmul inputs are bitcast to float32r (fast fp32 mode: 1 cycle/row at N>=256
vs 4 for strict fp32). Set MM_FAST=False for strict fp32 matmuls.

Biases (bq,bk,bv,bo) are zero and gamma/beta are one/zero for this problem's
setup_inputs; the wrapper checks that numerically. gamma/beta are applied on
the host when nontrivial (exact); nonzero biases fall back to a numpy path.
"""

import math
import sys

import numpy as np

sys.path.insert(0, "/opt/trn_rl_repo")

import concourse.bass as bass  # noqa: E402
from concourse import bacc  # noqa: E402
import concourse.tile as tile  # noqa: E402
from concourse import mybir  # noqa: E402
from concourse.masks import make_identity  # noqa: E402

B, S, H = 8, 2048, 768
P = 128
KT = H // P          # 6 k/h tiles
ST = S // P          # 16 t/s tiles
SCHUNK = 512
NSC = S // SCHUNK    # 4 s-chunks
INV_SQRT_H = 1.0 / math.sqrt(H)
EPS = 1e-12
MASK_NEG = -5.0e4    # exp(x + MASK_NEG) == 0.0 exactly in fp32 for |x| < 1e3
F32 = mybir.dt.float32
I32 = mybir.dt.int32

MM_FAST = True       # float32r matmuls (4x faster PE, slightly lower precision)


F32R = mybir.dt.float32r if MM_FAST else F32


def _mm(ap):
    return ap


def build_bass(tpad):
    """tpad: padded compacted KV length (multiple of 128; >=256-wide chunks)."""
    nc = bacc.Bacc("TRN2")
    TT = tpad // P                     # kv t-tiles
    # rhs chunk widths for K^T production over the kv axis (each >=256 for
    # full-rate fp32r; tpad is a multiple of 512, or a single 256/384 tail)
    kv_chunks = []
    off = 0
    while off < tpad:
        rem = tpad - off
        if rem % 512 == 128 and rem > 128:
            w = 384  # avoid a 128-wide tail (slow fp32r); use 384+256
        else:
            w = min(512, rem)
        kv_chunks.append((off, w))
        off += w

    hs = nc.dram_tensor("hidden_states", [S, H], F32, kind="ExternalInput")
    hkv = nc.dram_tensor("hidden_kv", [tpad, H], F32, kind="ExternalInput")
    am = nc.dram_tensor("attention_mask", [1, tpad], I32, kind="ExternalInput")
    wq = nc.dram_tensor("Wq", [H, H], F32, kind="ExternalInput")
    wk = nc.dram_tensor("Wk", [H, H], F32, kind="ExternalInput")
    wv = nc.dram_tensor("Wv", [H, H], F32, kind="ExternalInput")
    wo = nc.dram_tensor("Wo", [H, H], F32, kind="ExternalInput")
    out = nc.dram_tensor("out", [S, H], F32, kind="ExternalOutput")

    with tile.TileContext(nc) as tc:
        with (
            tc.tile_pool(name="consts", bufs=1) as consts,
            tc.tile_pool(name="kt", bufs=1) as kt_pool,
            tc.tile_pool(name="vt", bufs=1) as vt_pool,
            tc.tile_pool(name="qt", bufs=1) as qt_pool,
        ):
            identity = consts.tile([P, P], F32)
            make_identity(nc, identity)
            ones_f32 = consts.tile([P, 1], F32)
            nc.vector.memset(ones_f32, 1.0)
            eps_t = consts.tile([P, 1], F32)
            nc.vector.memset(eps_t, EPS)

            # compact mask [1,tpad] -> additive offsets [P, TT]
            mask_i = consts.tile([P, TT], I32)
            nc.sync.dma_start(
                out=mask_i, in_=am[0, :].rearrange("(tt p) -> p tt", p=P)
            )
            maskoff = consts.tile([P, TT], F32)
            nc.vector.tensor_copy(out=maskoff, in_=mask_i)
            nc.vector.tensor_scalar(
                out=maskoff,
                in0=maskoff,
                scalar1=1.0,
                scalar2=-MASK_NEG,
                op0=mybir.AluOpType.subtract,
                op1=mybir.AluOpType.mult,
            )

            kt_tiles = [kt_pool.tile([P, tpad], F32R, tag=f"kt{h}", name=f"kt{h}") for h in range(KT)]
            vt_tiles = [vt_pool.tile([P, H], F32R, tag=f"vt{t}", name=f"vt{t}") for t in range(TT)]
            qt_tiles = [qt_pool.tile([P, S], F32R, tag=f"qt{h}", name=f"qt{h}") for h in range(KT)]

            # ---- Q^T = Wq-lhsT @ X^T (resident), from full X ---------------
            # Interleaved per 512-column block: 4 X-tile transposes, then the
            # 6x6 Q^T matmuls for that block (keeps PE fed while DMA streams).
            with (
                tc.tile_pool(name="ypool", bufs=1) as ypool,
                tc.tile_pool(name="xload", bufs=6) as xload,
                tc.tile_pool(name="wpool", bufs=1) as wpool,
                tc.tile_pool(name="wstage", bufs=3) as wstage,
                tc.tile_pool(name="ypsum", bufs=3, space="PSUM") as ypsum,
                tc.tile_pool(name="bpsum", bufs=4, space="PSUM") as bpsum,
            ):
                y_tiles = [ypool.tile([P, S], F32R, tag=f"y{k}", name=f"y{k}") for k in range(KT)]
                wq_tiles = [wpool.tile([P, H], F32R, name=f"w{_k}", tag=f"w{_k}") for _k in range(KT)]
                for k in range(KT):
                    wst = wstage.tile([P, H], F32, name="wst", tag="wst")
                    nc.scalar.dma_start(out=wst, in_=wq[k * P : (k + 1) * P, :])
                    nc.vector.tensor_copy(out=wq_tiles[k], in_=wst)

                for sc in range(NSC):
                    ssl = slice(sc * SCHUNK, (sc + 1) * SCHUNK)
                    for st in range(sc * 4, sc * 4 + 4):
                        xt = xload.tile([P, H], F32, name="xt", tag="xt")
                        dma_eng = nc.sync if st % 2 == 0 else nc.scalar
                        dma_eng.dma_start(out=xt, in_=hs[st * P : (st + 1) * P, :])
                        for k in range(KT):
                            tp = ypsum.tile([P, P], F32, name="tp")
                            nc.tensor.transpose(
                                tp, xt[:, k * P : (k + 1) * P], identity
                            )
                            nc.vector.tensor_copy(
                                out=y_tiles[k][:, st * P : (st + 1) * P], in_=tp
                            )
                    for h in range(KT):
                        pp = bpsum.tile([P, SCHUNK], F32, name="pp", tag="bpsum")
                        for k in range(KT):
                            nc.tensor.matmul(
                                pp,
                                _mm(wq_tiles[k][:, h * P : (h + 1) * P]),
                                _mm(y_tiles[k][:, ssl]),
                                start=(k == 0),
                                stop=(k == KT - 1),
                            )
                        nc.scalar.copy(out=qt_tiles[h][:, ssl], in_=pp)

            # ---- K^T, V from compacted KV rows (interleaved) ---------------
            with (
                tc.tile_pool(name="ykv", bufs=1) as ykvpool,
                tc.tile_pool(name="xkload", bufs=6) as xkload,
                tc.tile_pool(name="wpool2", bufs=2) as wpool2,
                tc.tile_pool(name="wstage2", bufs=3) as wstage2,
                tc.tile_pool(name="ypsum2", bufs=3, space="PSUM") as ypsum2,
                tc.tile_pool(name="bpsum2", bufs=4, space="PSUM") as bpsum2,
            ):
                ykv_tiles = [ykvpool.tile([P, tpad], F32R, tag=f"yk{k}", name=f"yk{k}") for k in range(KT)]
                wk_tiles = [wpool2.tile([P, H], F32R, name=f"wk{_k}", tag=f"wk{_k}") for _k in range(KT)]
                for k in range(KT):
                    wst = wstage2.tile([P, H], F32, name="wst2", tag="wst2")
                    nc.gpsimd.dma_start(out=wst, in_=wk[k * P : (k + 1) * P, :])
                    nc.vector.tensor_copy(out=wk_tiles[k], in_=wst)
                wv_tiles = [wpool2.tile([P, H], F32R, name=f"wv{_k}", tag=f"wk{_k}") for _k in range(KT)]
                for k in range(KT):
                    wst = wstage2.tile([P, H], F32, name="wst2", tag="wst2")
                    nc.gpsimd.dma_start(out=wst, in_=wv[k * P : (k + 1) * P, :])
                    nc.vector.tensor_copy(out=wv_tiles[k], in_=wst)

                for off, w in kv_chunks:
                    t_lo, t_hi = off // P, (off + w) // P
                    for st in range(t_lo, t_hi):
                        xt = xkload.tile([P, H], F32, name="xkt", tag="xkt")
                        nc.sync.dma_start(out=xt, in_=hkv[st * P : (st + 1) * P, :])
                        for k in range(KT):
                            tp = ypsum2.tile([P, P], F32, name="tp2")
                            nc.tensor.transpose(
                                tp, xt[:, k * P : (k + 1) * P], identity
                            )
                            nc.vector.tensor_copy(
                                out=ykv_tiles[k][:, st * P : (st + 1) * P], in_=tp
                            )
                    for h in range(KT):
                        pp = bpsum2.tile([P, w], F32, name="pp2", tag="bpsum2")
                        for k in range(KT):
                            nc.tensor.matmul(
                                pp,
                                _mm(wk_tiles[k][:, h * P : (h + 1) * P]),
                                _mm(ykv_tiles[k][:, off : off + w]),
                                start=(k == 0),
                                stop=(k == KT - 1),
                            )
                        nc.scalar.copy(
                            out=kt_tiles[h][:, off : off + w], in_=pp
                        )
                    for t in range(t_lo, t_hi):
                        for n0, nw in ((0, 512), (512, 256)):
                            pp = bpsum2.tile([P, nw], F32, name="pp2", tag="bpsum2")
                            for k in range(KT):
                                nc.tensor.matmul(
                                    pp,
                                    _mm(ykv_tiles[k][:, t * P : (t + 1) * P]),
                                    _mm(wv_tiles[k][:, n0 : n0 + nw]),
                                    start=(k == 0),
                                    stop=(k == KT - 1),
                                )
                            nc.vector.tensor_copy(
                                out=vt_tiles[t][:, n0 : n0 + nw], in_=pp
                            )

            # ---- attention + output projection + LayerNorm -----------------
            with (
                tc.tile_pool(name="wopool", bufs=1) as wopool,
                tc.tile_pool(name="pt", bufs=3) as pt_pool,
                tc.tile_pool(name="zpool", bufs=2) as zpool,
                tc.tile_pool(name="ctxsb", bufs=1) as ctx_sb_pool,
                tc.tile_pool(name="xres", bufs=4) as xres_pool,
                tc.tile_pool(name="hbuf", bufs=2) as h_pool,
                tc.tile_pool(name="obuf", bufs=4) as out_pool,
                tc.tile_pool(name="stats", bufs=8) as st_pool,
                tc.tile_pool(name="cdpsum", bufs=6, space="PSUM") as cdpsum,
                tc.tile_pool(name="spsum", bufs=2, space="PSUM") as spsum,
            ):
                wo_tiles = [wopool.tile([P, H], F32R, name=f"wo{_k}", tag=f"wo{_k}") for _k in range(KT)]
                for k in range(KT):
                    wst = wopool.tile([P, H], F32, name="wost", tag="wost", bufs=2)
                    nc.gpsimd.dma_start(out=wst, in_=wo[k * P : (k + 1) * P, :])
                    nc.vector.tensor_copy(out=wo_tiles[k], in_=wst)

                for sc in range(NSC):
                    ssl = slice(sc * SCHUNK, (sc + 1) * SCHUNK)
                    n_stile = SCHUNK // P  # 4 s-tiles per chunk

                    zacc = zpool.tile([P, SCHUNK], F32, tag="zacc")
                    nc.vector.memset(zacc, 0.0)

                    ctx_psum = [
                        cdpsum.tile([P, SCHUNK], F32, tag="cd", name=f"ctxp{_h}") for _h in range(KT)
                    ]
                    pt_tiles = [None] * TT

                    def emit_s(t):
                        sp = spsum.tile([P, SCHUNK], F32, tag="sp", name="sp")
                        for h in range(KT):
                            nc.tensor.matmul(
                                sp,
                                _mm(kt_tiles[h][:, t * P : (t + 1) * P]),
                                _mm(qt_tiles[h][:, ssl]),
                                start=(h == 0),
                                stop=(h == KT - 1),
                            )
                        pt = pt_pool.tile([P, SCHUNK], F32R, name="pt")
                        nc.scalar.activation(
                            pt,
                            sp,
                            mybir.ActivationFunctionType.Exp,
                            bias=maskoff[:, t : t + 1],
                            scale=INV_SQRT_H,
                        )
                        nc.vector.tensor_add(zacc, zacc, pt)
                        pt_tiles[t] = pt

                    def emit_ctx(t):
                        pt = pt_tiles[t]
                        for h in range(KT):
                            nc.tensor.matmul(
                                ctx_psum[h],
                                _mm(vt_tiles[t][:, h * P : (h + 1) * P]),
                                _mm(pt),
                                start=(t == 0),
                                stop=(t == TT - 1),
                            )

                    emit_s(0)
                    for t in range(1, TT):
                        emit_s(t)
                        emit_ctx(t - 1)
                    emit_ctx(TT - 1)

                    zps = spsum.tile([P, n_stile], F32, tag="sp", name="zps")
                    for i in range(n_stile):
                        nc.tensor.matmul(
                            zps[:, i : i + 1],
                            zacc[:, i * P : (i + 1) * P],
                            ones_f32,
                            start=True,
                            stop=True,
                        )
                    zrec = zpool.tile([P, n_stile], F32, tag="zrec")
                    nc.vector.reciprocal(zrec, zps)

                    ctx_sb = []
                    for h in range(KT):
                        c = ctx_sb_pool.tile([P, SCHUNK], F32R, name=f"ctxsb{h}", tag=f"ctxsb{h}")
                        nc.vector.tensor_copy(out=c, in_=ctx_psum[h])
                        ctx_sb.append(c)

                    for i in range(n_stile):
                        srow = sc * SCHUNK + i * P
                        xr = xres_pool.tile([P, H], F32, name="xr")
                        nc.sync.dma_start(out=xr, in_=hs[srow : srow + P, :])
                        hsb = h_pool.tile([P, H], F32, name="hsb")
                        for n0, nw in ((0, 512), (512, 256)):
                            op = cdpsum.tile([P, nw], F32, tag="cd", name="op")
                            for h in range(KT):
                                nc.tensor.matmul(
                                    op,
                                    _mm(ctx_sb[h][:, i * P : (i + 1) * P]),
                                    _mm(wo_tiles[h][:, n0 : n0 + nw]),
                                    start=(h == 0),
                                    stop=(h == KT - 1),
                                )
                            # h = O * (1/Z) + X  (one DVE op, PSUM read)
                            nc.vector.scalar_tensor_tensor(
                                out=hsb[:, n0 : n0 + nw],
                                in0=op,
                                scalar=zrec[:, i : i + 1],
                                in1=xr[:, n0 : n0 + nw],
                                op0=mybir.AluOpType.mult,
                                op1=mybir.AluOpType.add,
                            )
                        # LayerNorm over H=768 (3 x 256 bn_stats groups)
                        stats = st_pool.tile([P, 3, 6], F32, tag="bn", name="stats")
                        for g in range(3):
                            nc.vector.bn_stats(
                                out=stats[:, g, :],
                                in_=hsb[:, g * 256 : (g + 1) * 256],
                            )
                        mv = st_pool.tile([P, 2], F32, tag="mv", name="mv")
                        nc.vector.bn_aggr(out=mv, in_=stats)
                        sd = st_pool.tile([P, 1], F32, tag="sd", name="sd")
                        nc.scalar.activation(
                            sd,
                            mv[:, 1:2],
                            mybir.ActivationFunctionType.Sqrt,
                            bias=eps_t,
                        )
                        nc.vector.reciprocal(sd, sd)
                        osb = out_pool.tile([P, H], F32, name="osb")
                        nc.vector.tensor_scalar(
                            out=osb,
                            in0=hsb,
                            scalar1=mv[:, 0:1],
                            scalar2=sd,
                            op0=mybir.AluOpType.subtract,
                            op1=mybir.AluOpType.mult,
                        )
                        nc.sync.dma_start(out=out[srow : srow + P, :], in_=osb)

    nc.finalize()
    return nc


_NC = {}


def _get_nc(tpad=None):
    if tpad is None:
        tpad = _DEFAULT_TPAD
    if tpad not in _NC:
        _NC[tpad] = build_bass(tpad)
    return _NC[tpad]


_DEFAULT_TPAD = 1280
_LAST_TPAD = _DEFAULT_TPAD


def _choose_tpad(nkeep_max):
    # multiple of 128; avoid a trailing K^T rhs chunk of 128 (slow fp32r):
    # tpad % 512 must be 0, 256, or 384
    t = max(256, ((nkeep_max + 127) // 128) * 128)
    return min(t, S)


def _compact_kv(hs_b, am_b, tpad):
    """Gather kept KV rows of one batch element, zero-padded to tpad."""
    idx = np.nonzero(am_b[0] != 0)[0]
    xkv = np.zeros((tpad, H), np.float32)
    xkv[: len(idx)] = hs_b[idx]
    mk = np.zeros((1, tpad), np.int32)
    mk[0, : len(idx)] = 1
    return xkv, mk


def _numpy_reference(hs, am, Wq, bq, Wk, bk, Wv, bv, Wo, bo, gamma, beta):
    q = hs @ Wq + bq
    k = hs @ Wk + bk
    v = hs @ Wv + bv
    scores = np.einsum("bsh,bth->bst", q, k) / math.sqrt(H)
    keep = am.astype(bool)  # [B,1,S]
    neg = np.finfo(np.float32).min
    masked = np.where(keep, scores, neg)
    m = masked.max(axis=-1, keepdims=True)
    e = np.exp(masked - m)
    probs = e / e.sum(axis=-1, keepdims=True)
    probs = np.where(keep, probs, 0.0)
    ctx = np.einsum("bst,bth->bsh", probs, v)
    h = ctx @ Wo + bo + hs
    mu = h.mean(-1, keepdims=True)
    var = ((h - mu) ** 2).mean(-1, keepdims=True)
    return ((h - mu) / np.sqrt(var + EPS) * gamma + beta).astype(np.float32)


def _run_device(in_maps, tpad, trace=False):
    from concourse.bass_utils import run_bass_kernel_spmd

    return run_bass_kernel_spmd(_get_nc(tpad), in_maps, list(range(B)), trace=trace)


def _make_in_maps(hs, am, Wq, Wk, Wv, Wo, tpad):
    maps = []
    for b in range(B):
        xkv, mk = _compact_kv(hs[b], am[b], tpad)
        maps.append(
            {
                "hidden_states": np.ascontiguousarray(hs[b]),
                "hidden_kv": xkv,
                "attention_mask": mk,
                "Wq": Wq,
                "Wk": Wk,
                "Wv": Wv,
                "Wo": Wo,
            }
        )
    return maps


def kernel(**inputs):
    hs = np.asarray(inputs["hidden_states"], dtype=np.float32)
    am = np.asarray(inputs["attention_mask"], dtype=np.int32)
    Wq = np.ascontiguousarray(np.asarray(inputs["Wq"], dtype=np.float32))
    Wk = np.ascontiguousarray(np.asarray(inputs["Wk"], dtype=np.float32))
    Wv = np.ascontiguousarray(np.asarray(inputs["Wv"], dtype=np.float32))
    Wo = np.ascontiguousarray(np.asarray(inputs["Wo"], dtype=np.float32))
    bq, bk = np.asarray(inputs["bq"]), np.asarray(inputs["bk"])
    bv, bo = np.asarray(inputs["bv"]), np.asarray(inputs["bo"])
    gamma, beta = np.asarray(inputs["gamma"]), np.asarray(inputs["beta"])

    nkeeps = [(am[b, 0] != 0).sum() for b in range(B)]
    if any(np.any(b != 0) for b in (bq, bk, bv, bo)) or min(nkeeps) == 0:
        # Never hit for this problem's setup_inputs (biases are zeros, masks
        # nonempty); exact-but-slow fallback for generality.
        return _numpy_reference(hs, am, Wq, bq, Wk, bk, Wv, bv, Wo, bo, gamma, beta)

    tpad = _choose_tpad(int(max(nkeeps)))
    global _LAST_TPAD
    _LAST_TPAD = tpad
    res = _run_device(_make_in_maps(hs, am, Wq, Wk, Wv, Wo, tpad), tpad)
    out = np.stack([res.results[b]["out"] for b in range(B)], axis=0)
    if np.any(gamma != 1):
        out = out * gamma.astype(np.float32)
    if np.any(beta != 0):
        out = out + beta.astype(np.float32)
    return np.ascontiguousarray(out.astype(np.float32))
